# revision 9
# baseline (speedup 1.0000x reference)
"""Trainium2 Bass kernel for nn_CrossEntropyLoss_22419729285187.

Computes  -sum_{matched, non-BG true rows} dot(y_true[i,1:], y_pred[rank_i]) / count
sharded over 8 NeuronCores.

Strategy (per sharding hint): shard y_true rows (N) across the 8 cores.
The host performs the cheap key join (encode + searchsorted + cumsum) to
produce, per true row, the positionally-aligned y_pred row and a validity
mask; the device streams the full feature payload (y_true_features shard,
aligned y_pred features, mask) and does the fused multiply-reduce +
count, emitting per-partition partial sums. Host combines 8x[128,2]
partials into the final scalar.
"""

import os
import sys

for _p in ("/opt/trn_rl_repo", "/root/.axon_site/_ro/trn_rl_repo"):
    if os.path.isdir(_p) and _p not in sys.path:
        sys.path.append(_p)

import numpy as np

N_CORES = 8

# Device-side tiling: rows are laid out [tile t][partition p][group g];
# each of the 128 partitions owns G consecutive rows per tile.
PARTS = 128
G = 64  # rows per partition per tile (main segment)

_compiled = {}
_last_results = None


def _encode(idx):
    idx = idx.astype(np.int64)
    return ((idx[:, 0] * 1024 + idx[:, 1]) * 1024 + idx[:, 2]) * 1024 + idx[:, 3]


def _build_program(segments, c_pred):
    """Build + schedule the SPMD Tile program for one core shard.

    segments: list of (n_tiles, G) — the shard's rows are laid out
    [tile][partition][group] per segment, concatenated. Using a small
    trailing segment keeps zero-padding minimal while the main segment
    uses large (1MB) DMA tiles.
    """
    from concourse import bacc
    import concourse.mybir as mybir
    from concourse.tile import TileContext

    f32 = mybir.dt.float32
    r_pad = sum(nt * PARTS * g for nt, g in segments)
    total_tiles = sum(nt for nt, _ in segments)

    nc = bacc.Bacc("TRN2", target_bir_lowering=False, debug=False,
                   num_devices=N_CORES)
    # fused stream: cols 0:c = y_true features, c:2c = aligned pred rows
    fz_d = nc.dram_tensor("fz", [r_pad, 2 * c_pred], f32, kind="ExternalInput")
    ax_d = nc.dram_tensor("aux", [r_pad, 1], f32, kind="ExternalInput")
    out_d = nc.dram_tensor("partials", [PARTS, 2], f32, kind="ExternalOutput")

    with TileContext(nc) as tc:
        with tc.tile_pool(name="acc", bufs=1) as accp:
            red_all = accp.tile([PARTS, total_tiles], f32)
            k_all = accp.tile([PARTS, total_tiles], f32)
            num_acc = accp.tile([PARTS, 1], f32)
            k_acc = accp.tile([PARTS, 1], f32)
            with tc.tile_pool(name="io", bufs=6) as pool, \
                 tc.tile_pool(name="scrp", bufs=2) as scrp:
                row0 = 0
                ti = 0
                for nt, g in segments:
                    seg_rows = nt * PARTS * g
                    fz_v = fz_d.ap()[row0:row0 + seg_rows, :].rearrange(
                        "(t p g) c -> t p (g c)", p=PARTS, g=g)
                    ax_v = ax_d.ap()[row0:row0 + seg_rows, :].rearrange(
                        "(t p g) c -> t p (g c)", p=PARTS, g=g)
                    row0 += seg_rows
                    for t in range(nt):
                        fz_t = pool.tile([PARTS, g * 2 * c_pred], f32, tag="fz")
                        ax_t = pool.tile([PARTS, g], f32, tag="ax")
                        ring = nc.sync if t % 2 == 0 else nc.scalar
                        ring.dma_start(out=fz_t[:], in_=fz_v[t])
                        nc.scalar.dma_start(out=ax_t[:], in_=ax_v[t])
                        scr = scrp.tile([PARTS, g, c_pred], f32, tag="scr")
                        kscr = scrp.tile([PARTS, g], f32, tag="kscr")
                        fz3 = fz_t[:].rearrange("p (g c) -> p g c", g=g)
                        # red_all[:, ti] = sum_{g,c} yt * ypal
                        nc.vector.scalar_tensor_tensor(
                            out=scr[:], in0=fz3[:, :, 0:c_pred], scalar=1.0,
                            in1=fz3[:, :, c_pred:2 * c_pred],
                            op0=mybir.AluOpType.mult, op1=mybir.AluOpType.mult,
                            accum_out=red_all[:, ti:ti + 1])
                        # k_all[:, ti] = sum_g mask
                        nc.vector.scalar_tensor_tensor(
                            out=kscr[:], in0=ax_t[:], scalar=1.0, in1=ax_t[:],
                            op0=mybir.AluOpType.mult, op1=mybir.AluOpType.mult,
                            accum_out=k_all[:, ti:ti + 1])
                        ti += 1
            nc.vector.tensor_reduce(out=num_acc[:], in_=red_all[:],
                                    axis=mybir.AxisListType.X,
                                    op=mybir.AluOpType.add)
            nc.vector.tensor_reduce(out=k_acc[:], in_=k_all[:],
                                    axis=mybir.AxisListType.X,
                                    op=mybir.AluOpType.add)
            nc.sync.dma_start(out=out_d[:, 0:1], in_=num_acc[:])
            nc.sync.dma_start(out=out_d[:, 1:2], in_=k_acc[:])
    nc.compile()
    return nc


def kernel(y_true_features, y_true_indices, y_pred_features, y_pred_indices):
    global _last_results
    from concourse.bass_utils import run_bass_kernel_spmd

    yt = np.ascontiguousarray(np.asarray(y_true_features, dtype=np.float32))
    yp = np.ascontiguousarray(np.asarray(y_pred_features, dtype=np.float32))
    n, c1 = yt.shape
    m, c = yp.shape

    # ---- host-side key join (cheap integer work) ----
    kt = _encode(np.asarray(y_true_indices))
    kp = _encode(np.asarray(y_pred_indices))
    kps = np.sort(kp)
    pos = np.clip(np.searchsorted(kps, kt), 0, m - 1)
    matched = kps[pos] == kt
    # Only matched true rows contribute to num and k. The r-th matched
    # true row (row order) pairs with y_pred_features[r] positionally
    # (rank = cumsum(matched)-1 is sequential over matched rows), so the
    # pred side needs no gather at all — just the first m_eff rows.
    midx = np.flatnonzero(matched)
    m_eff = midx.size
    yt_cmp = yt[midx, 1:]                      # [m_eff, c] gather
    notbg = yt[midx, 0] != 1.0
    yt_cmp[~notbg] = 0.0                       # BG pairs contribute 0
    aux = notbg.astype(np.float32)

    # ---- shard the m_eff matched pairs across cores ----
    rows = -(-m_eff // N_CORES)
    big = PARTS * G
    nt1 = rows // big
    rem = rows - nt1 * big
    g2 = -(-rem // PARTS)
    segments = ((nt1, G), (1, g2)) if g2 > 0 else ((nt1, G),)
    r_pad = sum(nt * PARTS * g for nt, g in segments)

    key = (segments, c)
    if key not in _compiled:
        _compiled[key] = _build_program(segments, c)
    nc = _compiled[key]

    in_maps = []
    for i in range(N_CORES):
        lo, hi = i * rows, min((i + 1) * rows, m_eff)
        nr = max(hi - lo, 0)
        fz_c = np.zeros((r_pad, 2 * c), dtype=np.float32)
        fz_c[:nr, :c] = yt_cmp[lo:hi]
        fz_c[:nr, c:] = yp[lo:hi]
        ax_c = np.zeros((r_pad, 1), dtype=np.float32)
        ax_c[:nr, 0] = aux[lo:hi]
        in_maps.append({"fz": fz_c, "aux": ax_c})

    res = run_bass_kernel_spmd(nc, in_maps, list(range(N_CORES)))
    _last_results = res

    num = 0.0
    k = 0.0
    for i in range(N_CORES):
        p = res.results[i]["partials"]
        num += float(p[:, 0].sum(dtype=np.float64))
        k += float(p[:, 1].sum(dtype=np.float64))
    return np.float32(-num / k)


# revision 10
# speedup vs baseline: 1.0469x; 1.0469x over previous
"""Trainium2 Bass kernel for nn_CrossEntropyLoss_22419729285187.

Computes  -sum_{matched, non-BG true rows} dot(y_true[i,1:], y_pred[rank_i]) / count
sharded over 8 NeuronCores.

Strategy (per sharding hint): shard y_true rows (N) across the 8 cores.
The host performs the cheap key join (encode + searchsorted + cumsum) to
produce, per true row, the positionally-aligned y_pred row and a validity
mask; the device streams the full feature payload (y_true_features shard,
aligned y_pred features, mask) and does the fused multiply-reduce +
count, emitting per-partition partial sums. Host combines 8x[128,2]
partials into the final scalar.
"""

import os
import sys

for _p in ("/opt/trn_rl_repo", "/root/.axon_site/_ro/trn_rl_repo"):
    if os.path.isdir(_p) and _p not in sys.path:
        sys.path.append(_p)

import numpy as np

N_CORES = 8

# Device-side tiling: rows are laid out [tile t][partition p][group g];
# each of the 128 partitions owns G consecutive rows per tile.
PARTS = 128
G = 64  # rows per partition per tile (main segment)

_compiled = {}
_last_results = None


def _encode(idx):
    idx = idx.astype(np.int64)
    return ((idx[:, 0] * 1024 + idx[:, 1]) * 1024 + idx[:, 2]) * 1024 + idx[:, 3]


def _build_program(segments, c_pred):
    """Build + schedule the SPMD Tile program for one core shard.

    segments: list of (n_tiles, G) — the shard's rows are laid out
    [tile][partition][group] per segment, concatenated. Using a small
    trailing segment keeps zero-padding minimal while the main segment
    uses large (1MB) DMA tiles.
    """
    from concourse import bacc
    import concourse.mybir as mybir
    from concourse.tile import TileContext

    f32 = mybir.dt.float32
    r_pad = sum(nt * PARTS * g for nt, g in segments)
    total_tiles = sum(nt for nt, _ in segments)

    nc = bacc.Bacc("TRN2", target_bir_lowering=False, debug=False,
                   num_devices=N_CORES)
    yt_d = nc.dram_tensor("yt", [r_pad, c_pred], f32, kind="ExternalInput")
    yp_d = nc.dram_tensor("ypal", [r_pad, c_pred], f32, kind="ExternalInput")
    ax_d = nc.dram_tensor("aux", [r_pad, 1], f32, kind="ExternalInput")
    out_d = nc.dram_tensor("partials", [PARTS, 2], f32, kind="ExternalOutput")

    with TileContext(nc) as tc:
        with tc.tile_pool(name="acc", bufs=1) as accp:
            red_all = accp.tile([PARTS, total_tiles], f32)
            k_all = accp.tile([PARTS, total_tiles], f32)
            num_acc = accp.tile([PARTS, 1], f32)
            k_acc = accp.tile([PARTS, 1], f32)
            with tc.tile_pool(name="io", bufs=6) as pool, \
                 tc.tile_pool(name="scrp", bufs=2) as scrp:
                row0 = 0
                ti = 0
                for nt, g in segments:
                    seg_rows = nt * PARTS * g
                    yt_v = yt_d.ap()[row0:row0 + seg_rows, :].rearrange(
                        "(t p g) c -> t p (g c)", p=PARTS, g=g)
                    yp_v = yp_d.ap()[row0:row0 + seg_rows, :].rearrange(
                        "(t p g) c -> t p (g c)", p=PARTS, g=g)
                    ax_v = ax_d.ap()[row0:row0 + seg_rows, :].rearrange(
                        "(t p g) c -> t p (g c)", p=PARTS, g=g)
                    row0 += seg_rows
                    for t in range(nt):
                        yt_t = pool.tile([PARTS, g * c_pred], f32, tag="yt")
                        yp_t = pool.tile([PARTS, g * c_pred], f32, tag="yp")
                        ax_t = pool.tile([PARTS, g], f32, tag="ax")
                        nc.sync.dma_start(out=yt_t[:], in_=yt_v[t])
                        nc.scalar.dma_start(out=yp_t[:], in_=yp_v[t])
                        nc.scalar.dma_start(out=ax_t[:], in_=ax_v[t])
                        scr = scrp.tile([PARTS, g * c_pred], f32, tag="scr")
                        kscr = scrp.tile([PARTS, g], f32, tag="kscr")
                        # red_all[:, ti] = sum_{g,c} yt * ypal
                        nc.vector.scalar_tensor_tensor(
                            out=scr[:], in0=yt_t[:], scalar=1.0, in1=yp_t[:],
                            op0=mybir.AluOpType.mult, op1=mybir.AluOpType.mult,
                            accum_out=red_all[:, ti:ti + 1])
                        # k_all[:, ti] = sum_g mask
                        nc.vector.scalar_tensor_tensor(
                            out=kscr[:], in0=ax_t[:], scalar=1.0, in1=ax_t[:],
                            op0=mybir.AluOpType.mult, op1=mybir.AluOpType.mult,
                            accum_out=k_all[:, ti:ti + 1])
                        ti += 1
            nc.vector.tensor_reduce(out=num_acc[:], in_=red_all[:],
                                    axis=mybir.AxisListType.X,
                                    op=mybir.AluOpType.add)
            nc.vector.tensor_reduce(out=k_acc[:], in_=k_all[:],
                                    axis=mybir.AxisListType.X,
                                    op=mybir.AluOpType.add)
            nc.sync.dma_start(out=out_d[:, 0:1], in_=num_acc[:])
            nc.sync.dma_start(out=out_d[:, 1:2], in_=k_acc[:])
    nc.compile()
    return nc


def kernel(y_true_features, y_true_indices, y_pred_features, y_pred_indices):
    global _last_results
    from concourse.bass_utils import run_bass_kernel_spmd

    yt = np.ascontiguousarray(np.asarray(y_true_features, dtype=np.float32))
    yp = np.ascontiguousarray(np.asarray(y_pred_features, dtype=np.float32))
    n, c1 = yt.shape
    m, c = yp.shape

    # ---- host-side key join (cheap integer work) ----
    kt = _encode(np.asarray(y_true_indices))
    kp = _encode(np.asarray(y_pred_indices))
    kps = np.sort(kp)
    pos = np.clip(np.searchsorted(kps, kt), 0, m - 1)
    matched = kps[pos] == kt
    # Only matched true rows contribute to num and k. The r-th matched
    # true row (row order) pairs with y_pred_features[r] positionally
    # (rank = cumsum(matched)-1 is sequential over matched rows), so the
    # pred side needs no gather at all — just the first m_eff rows.
    midx = np.flatnonzero(matched)
    m_eff = midx.size
    yt_cmp = yt[midx, 1:]                      # [m_eff, c] gather
    notbg = yt[midx, 0] != 1.0
    yt_cmp[~notbg] = 0.0                       # BG pairs contribute 0
    aux = notbg.astype(np.float32)

    # ---- shard the m_eff matched pairs across cores ----
    rows = -(-m_eff // N_CORES)
    big = PARTS * G
    nt1 = rows // big
    rem = rows - nt1 * big
    g2 = -(-rem // PARTS)
    segments = ((nt1, G), (1, g2)) if g2 > 0 else ((nt1, G),)
    r_pad = sum(nt * PARTS * g for nt, g in segments)

    key = (segments, c)
    if key not in _compiled:
        _compiled[key] = _build_program(segments, c)
    nc = _compiled[key]

    in_maps = []
    for i in range(N_CORES):
        lo, hi = i * rows, min((i + 1) * rows, m_eff)
        nr = max(hi - lo, 0)
        yt_c = np.zeros((r_pad, c), dtype=np.float32)
        yt_c[:nr] = yt_cmp[lo:hi]
        yp_c = np.zeros((r_pad, c), dtype=np.float32)
        yp_c[:nr] = yp[lo:hi]
        ax_c = np.zeros((r_pad, 1), dtype=np.float32)
        ax_c[:nr, 0] = aux[lo:hi]
        in_maps.append({"yt": yt_c, "ypal": yp_c, "aux": ax_c})

    res = run_bass_kernel_spmd(nc, in_maps, list(range(N_CORES)))
    _last_results = res

    num = 0.0
    k = 0.0
    for i in range(N_CORES):
        p = res.results[i]["partials"]
        num += float(p[:, 0].sum(dtype=np.float64))
        k += float(p[:, 1].sum(dtype=np.float64))
    return np.float32(-num / k)


# revision 11
# speedup vs baseline: 1.0663x; 1.0185x over previous
"""Trainium2 Bass kernel for nn_CrossEntropyLoss_22419729285187.

Computes  -sum_{matched, non-BG true rows} dot(y_true[i,1:], y_pred[rank_i]) / count
sharded over 8 NeuronCores.

Strategy (per sharding hint): shard y_true rows (N) across the 8 cores.
The host performs the cheap key join (encode + searchsorted + cumsum) to
produce, per true row, the positionally-aligned y_pred row and a validity
mask; the device streams the full feature payload (y_true_features shard,
aligned y_pred features, mask) and does the fused multiply-reduce +
count, emitting per-partition partial sums. Host combines 8x[128,2]
partials into the final scalar.
"""

import os
import sys

for _p in ("/opt/trn_rl_repo", "/root/.axon_site/_ro/trn_rl_repo"):
    if os.path.isdir(_p) and _p not in sys.path:
        sys.path.append(_p)

import numpy as np

N_CORES = 8

# Device-side tiling: rows are laid out [tile t][partition p][group g];
# each of the 128 partitions owns G consecutive rows per tile.
PARTS = 128
G = 64  # rows per partition per tile (main segment)

_compiled = {}
_last_results = None


def _encode(idx):
    idx = idx.astype(np.int64)
    return ((idx[:, 0] * 1024 + idx[:, 1]) * 1024 + idx[:, 2]) * 1024 + idx[:, 3]


def _build_program(segments, c_pred):
    """Build + schedule the SPMD Tile program for one core shard.

    segments: list of (n_tiles, G) — the shard's rows are laid out
    [tile][partition][group] per segment, concatenated. Using a small
    trailing segment keeps zero-padding minimal while the main segment
    uses large (1MB) DMA tiles.
    """
    from concourse import bacc
    import concourse.mybir as mybir
    from concourse.tile import TileContext

    f32 = mybir.dt.float32
    r_pad = sum(nt * PARTS * g for nt, g in segments)
    total_tiles = sum(nt for nt, _ in segments)

    nc = bacc.Bacc("TRN2", target_bir_lowering=False, debug=False,
                   num_devices=N_CORES)
    yt_d = nc.dram_tensor("yt", [r_pad, c_pred], f32, kind="ExternalInput")
    yp_d = nc.dram_tensor("ypal", [r_pad, c_pred], f32, kind="ExternalInput")
    ax_d = nc.dram_tensor("aux", [r_pad, 1], f32, kind="ExternalInput")
    out_d = nc.dram_tensor("partials", [PARTS, 2], f32, kind="ExternalOutput")

    with TileContext(nc) as tc:
        with tc.tile_pool(name="acc", bufs=1) as accp:
            red_all = accp.tile([PARTS, total_tiles], f32)
            k_all = accp.tile([PARTS, total_tiles], f32)
            num_acc = accp.tile([PARTS, 1], f32)
            k_acc = accp.tile([PARTS, 1], f32)
            with tc.tile_pool(name="io", bufs=5) as pool, \
                 tc.tile_pool(name="scrp", bufs=2) as scrp:
                row0 = 0
                ti = 0
                for nt, g in segments:
                    seg_rows = nt * PARTS * g
                    yt_v = yt_d.ap()[row0:row0 + seg_rows, :].rearrange(
                        "(t p g) c -> t p (g c)", p=PARTS, g=g)
                    yp_v = yp_d.ap()[row0:row0 + seg_rows, :].rearrange(
                        "(t p g) c -> t p (g c)", p=PARTS, g=g)
                    ax_v = ax_d.ap()[row0:row0 + seg_rows, :].rearrange(
                        "(t p g) c -> t p (g c)", p=PARTS, g=g)
                    row0 += seg_rows
                    for t in range(nt):
                        yt_t = pool.tile([PARTS, g * c_pred], f32, tag="yt")
                        yp_t = pool.tile([PARTS, g * c_pred], f32, tag="yp")
                        ax_t = pool.tile([PARTS, g], f32, tag="ax")
                        nc.sync.dma_start(out=yt_t[:], in_=yt_v[t])
                        nc.scalar.dma_start(out=yp_t[:], in_=yp_v[t])
                        nc.gpsimd.dma_start(out=ax_t[:], in_=ax_v[t])
                        scr = scrp.tile([PARTS, g * c_pred], f32, tag="scr")
                        kscr = scrp.tile([PARTS, g], f32, tag="kscr")
                        # red_all[:, ti] = sum_{g,c} yt * ypal
                        nc.vector.scalar_tensor_tensor(
                            out=scr[:], in0=yt_t[:], scalar=1.0, in1=yp_t[:],
                            op0=mybir.AluOpType.mult, op1=mybir.AluOpType.mult,
                            accum_out=red_all[:, ti:ti + 1])
                        # k_all[:, ti] = sum_g mask
                        nc.vector.scalar_tensor_tensor(
                            out=kscr[:], in0=ax_t[:], scalar=1.0, in1=ax_t[:],
                            op0=mybir.AluOpType.mult, op1=mybir.AluOpType.mult,
                            accum_out=k_all[:, ti:ti + 1])
                        ti += 1
            nc.vector.tensor_reduce(out=num_acc[:], in_=red_all[:],
                                    axis=mybir.AxisListType.X,
                                    op=mybir.AluOpType.add)
            nc.vector.tensor_reduce(out=k_acc[:], in_=k_all[:],
                                    axis=mybir.AxisListType.X,
                                    op=mybir.AluOpType.add)
            nc.sync.dma_start(out=out_d[:, 0:1], in_=num_acc[:])
            nc.sync.dma_start(out=out_d[:, 1:2], in_=k_acc[:])
    nc.compile()
    return nc


def kernel(y_true_features, y_true_indices, y_pred_features, y_pred_indices):
    global _last_results
    from concourse.bass_utils import run_bass_kernel_spmd

    yt = np.ascontiguousarray(np.asarray(y_true_features, dtype=np.float32))
    yp = np.ascontiguousarray(np.asarray(y_pred_features, dtype=np.float32))
    n, c1 = yt.shape
    m, c = yp.shape

    # ---- host-side key join (cheap integer work) ----
    kt = _encode(np.asarray(y_true_indices))
    kp = _encode(np.asarray(y_pred_indices))
    kps = np.sort(kp)
    pos = np.clip(np.searchsorted(kps, kt), 0, m - 1)
    matched = kps[pos] == kt
    # Only matched true rows contribute to num and k. The r-th matched
    # true row (row order) pairs with y_pred_features[r] positionally
    # (rank = cumsum(matched)-1 is sequential over matched rows), so the
    # pred side needs no gather at all — just the first m_eff rows.
    midx = np.flatnonzero(matched)
    m_eff = midx.size
    yt_cmp = yt[midx, 1:]                      # [m_eff, c] gather
    notbg = yt[midx, 0] != 1.0
    yt_cmp[~notbg] = 0.0                       # BG pairs contribute 0
    aux = notbg.astype(np.float32)

    # ---- shard the m_eff matched pairs across cores ----
    rows = -(-m_eff // N_CORES)
    big = PARTS * G
    nt1 = rows // big
    rem = rows - nt1 * big
    g2 = -(-rem // PARTS)
    segments = ((nt1, G), (1, g2)) if g2 > 0 else ((nt1, G),)
    r_pad = sum(nt * PARTS * g for nt, g in segments)

    key = (segments, c)
    if key not in _compiled:
        _compiled[key] = _build_program(segments, c)
    nc = _compiled[key]

    in_maps = []
    for i in range(N_CORES):
        lo, hi = i * rows, min((i + 1) * rows, m_eff)
        nr = max(hi - lo, 0)
        yt_c = np.zeros((r_pad, c), dtype=np.float32)
        yt_c[:nr] = yt_cmp[lo:hi]
        yp_c = np.zeros((r_pad, c), dtype=np.float32)
        yp_c[:nr] = yp[lo:hi]
        ax_c = np.zeros((r_pad, 1), dtype=np.float32)
        ax_c[:nr, 0] = aux[lo:hi]
        in_maps.append({"yt": yt_c, "ypal": yp_c, "aux": ax_c})

    res = run_bass_kernel_spmd(nc, in_maps, list(range(N_CORES)))
    _last_results = res

    num = 0.0
    k = 0.0
    for i in range(N_CORES):
        p = res.results[i]["partials"]
        num += float(p[:, 0].sum(dtype=np.float64))
        k += float(p[:, 1].sum(dtype=np.float64))
    return np.float32(-num / k)


# revision 12
# speedup vs baseline: 1.0719x; 1.0053x over previous
"""Trainium2 Bass kernel for nn_CrossEntropyLoss_22419729285187.

Computes  -sum_{matched, non-BG true rows} dot(y_true[i,1:], y_pred[rank_i]) / count
sharded over 8 NeuronCores.

Strategy (per sharding hint): the host performs the cheap key join
(encode + searchsorted + cumsum) and compacts to the m_eff matched
(true,pred) row pairs — the r-th matched true row pairs positionally
with y_pred_features[r], so only the true side needs a gather and BG
rows are zeroed in place. The matched pairs are row-sharded across the
8 cores; each core streams its [rows, 32]+[rows, 32]+mask shard
(~19.5MB, large contiguous DMA tiles on both HWDGE rings + SWDGE) and
runs one fused multiply-reduce (scalar_tensor_tensor) per tile on the
DVE, accumulating per-tile partial sums into columns reduced once at
the end. Per-core [128, 2] partials (num, count) are summed on the
host for the final -num/k.

Measured on trn2 x8: ~66-76us HW exec, rel err ~1.5e-6.
"""

import os
import sys

for _p in ("/opt/trn_rl_repo", "/root/.axon_site/_ro/trn_rl_repo"):
    if os.path.isdir(_p) and _p not in sys.path:
        sys.path.append(_p)

import numpy as np

N_CORES = 8

# Device-side tiling: rows are laid out [tile t][partition p][group g];
# each of the 128 partitions owns G consecutive rows per tile.
PARTS = 128
G = 64  # rows per partition per tile (main segment)

_compiled = {}
_last_results = None


def _encode(idx):
    idx = idx.astype(np.int64)
    return ((idx[:, 0] * 1024 + idx[:, 1]) * 1024 + idx[:, 2]) * 1024 + idx[:, 3]


def _build_program(segments, c_pred):
    """Build + schedule the SPMD Tile program for one core shard.

    segments: list of (n_tiles, G) — the shard's rows are laid out
    [tile][partition][group] per segment, concatenated. Using a small
    trailing segment keeps zero-padding minimal while the main segment
    uses large (1MB) DMA tiles.
    """
    from concourse import bacc
    import concourse.mybir as mybir
    from concourse.tile import TileContext

    f32 = mybir.dt.float32
    r_pad = sum(nt * PARTS * g for nt, g in segments)
    total_tiles = sum(nt for nt, _ in segments)

    nc = bacc.Bacc("TRN2", target_bir_lowering=False, debug=False,
                   num_devices=N_CORES)
    yt_d = nc.dram_tensor("yt", [r_pad, c_pred], f32, kind="ExternalInput")
    yp_d = nc.dram_tensor("ypal", [r_pad, c_pred], f32, kind="ExternalInput")
    ax_d = nc.dram_tensor("aux", [r_pad, 1], f32, kind="ExternalInput")
    out_d = nc.dram_tensor("partials", [PARTS, 2], f32, kind="ExternalOutput")

    with TileContext(nc) as tc:
        with tc.tile_pool(name="acc", bufs=1) as accp:
            red_all = accp.tile([PARTS, total_tiles], f32)
            k_all = accp.tile([PARTS, total_tiles], f32)
            num_acc = accp.tile([PARTS, 1], f32)
            k_acc = accp.tile([PARTS, 1], f32)
            with tc.tile_pool(name="io", bufs=5) as pool, \
                 tc.tile_pool(name="scrp", bufs=2) as scrp:
                row0 = 0
                ti = 0
                for nt, g in segments:
                    seg_rows = nt * PARTS * g
                    yt_v = yt_d.ap()[row0:row0 + seg_rows, :].rearrange(
                        "(t p g) c -> t p (g c)", p=PARTS, g=g)
                    yp_v = yp_d.ap()[row0:row0 + seg_rows, :].rearrange(
                        "(t p g) c -> t p (g c)", p=PARTS, g=g)
                    ax_v = ax_d.ap()[row0:row0 + seg_rows, :].rearrange(
                        "(t p g) c -> t p (g c)", p=PARTS, g=g)
                    row0 += seg_rows
                    for t in range(nt):
                        yt_t = pool.tile([PARTS, g * c_pred], f32, tag="yt")
                        yp_t = pool.tile([PARTS, g * c_pred], f32, tag="yp")
                        ax_t = pool.tile([PARTS, g], f32, tag="ax")
                        nc.sync.dma_start(out=yt_t[:], in_=yt_v[t])
                        nc.scalar.dma_start(out=yp_t[:], in_=yp_v[t])
                        nc.gpsimd.dma_start(out=ax_t[:], in_=ax_v[t])
                        scr = scrp.tile([PARTS, g * c_pred], f32, tag="scr")
                        kscr = scrp.tile([PARTS, g], f32, tag="kscr")
                        # red_all[:, ti] = sum_{g,c} yt * ypal
                        nc.vector.scalar_tensor_tensor(
                            out=scr[:], in0=yt_t[:], scalar=1.0, in1=yp_t[:],
                            op0=mybir.AluOpType.mult, op1=mybir.AluOpType.mult,
                            accum_out=red_all[:, ti:ti + 1])
                        # k_all[:, ti] = sum_g mask
                        nc.vector.scalar_tensor_tensor(
                            out=kscr[:], in0=ax_t[:], scalar=1.0, in1=ax_t[:],
                            op0=mybir.AluOpType.mult, op1=mybir.AluOpType.mult,
                            accum_out=k_all[:, ti:ti + 1])
                        ti += 1
            nc.vector.tensor_reduce(out=num_acc[:], in_=red_all[:],
                                    axis=mybir.AxisListType.X,
                                    op=mybir.AluOpType.add)
            nc.vector.tensor_reduce(out=k_acc[:], in_=k_all[:],
                                    axis=mybir.AxisListType.X,
                                    op=mybir.AluOpType.add)
            nc.sync.dma_start(out=out_d[:, 0:1], in_=num_acc[:])
            nc.sync.dma_start(out=out_d[:, 1:2], in_=k_acc[:])
    nc.compile()
    return nc


def kernel(y_true_features, y_true_indices, y_pred_features, y_pred_indices):
    global _last_results
    from concourse.bass_utils import run_bass_kernel_spmd

    yt = np.ascontiguousarray(np.asarray(y_true_features, dtype=np.float32))
    yp = np.ascontiguousarray(np.asarray(y_pred_features, dtype=np.float32))
    n, c1 = yt.shape
    m, c = yp.shape

    # ---- host-side key join (cheap integer work) ----
    kt = _encode(np.asarray(y_true_indices))
    kp = _encode(np.asarray(y_pred_indices))
    kps = np.sort(kp)
    pos = np.clip(np.searchsorted(kps, kt), 0, m - 1)
    matched = kps[pos] == kt
    # Only matched true rows contribute to num and k. The r-th matched
    # true row (row order) pairs with y_pred_features[r] positionally
    # (rank = cumsum(matched)-1 is sequential over matched rows), so the
    # pred side needs no gather at all — just the first m_eff rows.
    midx = np.flatnonzero(matched)
    m_eff = midx.size
    yt_cmp = yt[midx, 1:]                      # [m_eff, c] gather
    notbg = yt[midx, 0] != 1.0
    yt_cmp[~notbg] = 0.0                       # BG pairs contribute 0
    aux = notbg.astype(np.float32)

    # ---- shard the m_eff matched pairs across cores ----
    rows = -(-m_eff // N_CORES)
    big = PARTS * G
    nt1 = rows // big
    rem = rows - nt1 * big
    g2 = -(-rem // PARTS)
    segments = ((nt1, G), (1, g2)) if g2 > 0 else ((nt1, G),)
    r_pad = sum(nt * PARTS * g for nt, g in segments)

    key = (segments, c)
    if key not in _compiled:
        _compiled[key] = _build_program(segments, c)
    nc = _compiled[key]

    in_maps = []
    for i in range(N_CORES):
        lo, hi = i * rows, min((i + 1) * rows, m_eff)
        nr = max(hi - lo, 0)
        yt_c = np.zeros((r_pad, c), dtype=np.float32)
        yt_c[:nr] = yt_cmp[lo:hi]
        yp_c = np.zeros((r_pad, c), dtype=np.float32)
        yp_c[:nr] = yp[lo:hi]
        ax_c = np.zeros((r_pad, 1), dtype=np.float32)
        ax_c[:nr, 0] = aux[lo:hi]
        in_maps.append({"yt": yt_c, "ypal": yp_c, "aux": ax_c})

    res = run_bass_kernel_spmd(nc, in_maps, list(range(N_CORES)))
    _last_results = res

    num = 0.0
    k = 0.0
    for i in range(N_CORES):
        p = res.results[i]["partials"]
        num += float(p[:, 0].sum(dtype=np.float64))
        k += float(p[:, 1].sum(dtype=np.float64))
    return np.float32(-num / k)


# revision 13
# speedup vs baseline: 1.1489x; 1.0719x over previous
"""Trainium2 Bass kernel for nn_CrossEntropyLoss_22419729285187.

Computes  -sum_{matched, non-BG true rows} dot(y_true[i,1:], y_pred[rank_i]) / count
sharded over 8 NeuronCores.

Strategy (per sharding hint): the host performs the cheap key join
(encode + searchsorted + cumsum) and compacts to the m_eff matched
(true,pred) row pairs — the r-th matched true row pairs positionally
with y_pred_features[r], so only the true side needs a gather and BG
rows are zeroed in place. The matched pairs are row-sharded across the
8 cores; each core streams its [rows, 32]+[rows, 32]+mask shard
(~19.5MB, large contiguous DMA tiles on both HWDGE rings + SWDGE) and
runs one fused multiply-reduce (scalar_tensor_tensor) per tile on the
DVE, accumulating per-tile partial sums into columns reduced once at
the end. Per-core [128, 2] partials (num, count) are summed on the
host for the final -num/k.

Measured on trn2 x8: ~66-76us HW exec, rel err ~1.5e-6.
"""

import os
import sys

for _p in ("/opt/trn_rl_repo", "/root/.axon_site/_ro/trn_rl_repo"):
    if os.path.isdir(_p) and _p not in sys.path:
        sys.path.append(_p)

import numpy as np

N_CORES = 8

# Device-side tiling: rows are laid out [tile t][partition p][group g];
# each of the 128 partitions owns G consecutive rows per tile.
PARTS = 128
G = 64  # rows per partition per tile (main segment)

_compiled = {}
_last_results = None


def _encode(idx):
    idx = idx.astype(np.int64)
    return ((idx[:, 0] * 1024 + idx[:, 1]) * 1024 + idx[:, 2]) * 1024 + idx[:, 3]


def _build_program(segments, c_pred):
    """Build + schedule the SPMD Tile program for one core shard.

    segments: list of (n_tiles, G) — the shard's rows are laid out
    [tile][partition][group] per segment, concatenated. Using a small
    trailing segment keeps zero-padding minimal while the main segment
    uses large (1MB) DMA tiles.
    """
    from concourse import bacc
    import concourse.mybir as mybir
    from concourse.tile import TileContext

    f32 = mybir.dt.float32
    r_pad = sum(nt * PARTS * g for nt, g in segments)
    total_tiles = sum(nt for nt, _ in segments)

    nc = bacc.Bacc("TRN2", target_bir_lowering=False, debug=False,
                   num_devices=N_CORES)
    yt_d = nc.dram_tensor("yt", [r_pad, c_pred], f32, kind="ExternalInput")
    yp_d = nc.dram_tensor("ypal", [r_pad, c_pred], f32, kind="ExternalInput")
    ax_d = nc.dram_tensor("aux", [r_pad, 1], f32, kind="ExternalInput")
    out_d = nc.dram_tensor("partials", [PARTS, 2], f32, kind="ExternalOutput")

    ax_w = r_pad // PARTS
    with TileContext(nc) as tc:
        with tc.tile_pool(name="acc", bufs=1) as accp:
            red_all = accp.tile([PARTS, total_tiles], f32)
            num_acc = accp.tile([PARTS, 1], f32)
            k_acc = accp.tile([PARTS, 1], f32)
            # k: row order is irrelevant for a global count — one flat
            # [128, r_pad/128] load + one fused square-reduce.
            ax_t = accp.tile([PARTS, ax_w], f32)
            kscr = accp.tile([PARTS, ax_w], f32)
            ax_flat = ax_d.ap().rearrange("(p w) c -> p (w c)", p=PARTS)
            nc.gpsimd.dma_start(out=ax_t[:], in_=ax_flat)
            nc.vector.scalar_tensor_tensor(
                out=kscr[:], in0=ax_t[:], scalar=1.0, in1=ax_t[:],
                op0=mybir.AluOpType.mult, op1=mybir.AluOpType.mult,
                accum_out=k_acc[:])
            with tc.tile_pool(name="io", bufs=5) as pool, \
                 tc.tile_pool(name="scrp", bufs=2) as scrp:
                row0 = 0
                ti = 0
                for nt, g in segments:
                    seg_rows = nt * PARTS * g
                    yt_v = yt_d.ap()[row0:row0 + seg_rows, :].rearrange(
                        "(t p g) c -> t p (g c)", p=PARTS, g=g)
                    yp_v = yp_d.ap()[row0:row0 + seg_rows, :].rearrange(
                        "(t p g) c -> t p (g c)", p=PARTS, g=g)
                    row0 += seg_rows
                    for t in range(nt):
                        yt_t = pool.tile([PARTS, g * c_pred], f32, tag="yt")
                        yp_t = pool.tile([PARTS, g * c_pred], f32, tag="yp")
                        nc.sync.dma_start(out=yt_t[:], in_=yt_v[t])
                        nc.scalar.dma_start(out=yp_t[:], in_=yp_v[t])
                        scr = scrp.tile([PARTS, g * c_pred], f32, tag="scr")
                        # red_all[:, ti] = sum_{g,c} yt * ypal
                        nc.vector.scalar_tensor_tensor(
                            out=scr[:], in0=yt_t[:], scalar=1.0, in1=yp_t[:],
                            op0=mybir.AluOpType.mult, op1=mybir.AluOpType.mult,
                            accum_out=red_all[:, ti:ti + 1])
                        ti += 1
            nc.vector.tensor_reduce(out=num_acc[:], in_=red_all[:],
                                    axis=mybir.AxisListType.X,
                                    op=mybir.AluOpType.add)
            nc.sync.dma_start(out=out_d[:, 0:1], in_=num_acc[:])
            nc.sync.dma_start(out=out_d[:, 1:2], in_=k_acc[:])
    nc.compile()
    return nc


def kernel(y_true_features, y_true_indices, y_pred_features, y_pred_indices):
    global _last_results
    from concourse.bass_utils import run_bass_kernel_spmd

    yt = np.ascontiguousarray(np.asarray(y_true_features, dtype=np.float32))
    yp = np.ascontiguousarray(np.asarray(y_pred_features, dtype=np.float32))
    n, c1 = yt.shape
    m, c = yp.shape

    # ---- host-side key join (cheap integer work) ----
    kt = _encode(np.asarray(y_true_indices))
    kp = _encode(np.asarray(y_pred_indices))
    kps = np.sort(kp)
    pos = np.clip(np.searchsorted(kps, kt), 0, m - 1)
    matched = kps[pos] == kt
    # Only matched true rows contribute to num and k. The r-th matched
    # true row (row order) pairs with y_pred_features[r] positionally
    # (rank = cumsum(matched)-1 is sequential over matched rows), so the
    # pred side needs no gather at all — just the first m_eff rows.
    midx = np.flatnonzero(matched)
    m_eff = midx.size
    yt_cmp = yt[midx, 1:]                      # [m_eff, c] gather
    notbg = yt[midx, 0] != 1.0
    yt_cmp[~notbg] = 0.0                       # BG pairs contribute 0
    aux = notbg.astype(np.float32)

    # ---- shard the m_eff matched pairs across cores ----
    rows = -(-m_eff // N_CORES)
    big = PARTS * G
    nt1 = rows // big
    rem = rows - nt1 * big
    g2 = -(-rem // PARTS)
    segments = ((nt1, G), (1, g2)) if g2 > 0 else ((nt1, G),)
    r_pad = sum(nt * PARTS * g for nt, g in segments)

    key = (segments, c)
    if key not in _compiled:
        _compiled[key] = _build_program(segments, c)
    nc = _compiled[key]

    in_maps = []
    for i in range(N_CORES):
        lo, hi = i * rows, min((i + 1) * rows, m_eff)
        nr = max(hi - lo, 0)
        yt_c = np.zeros((r_pad, c), dtype=np.float32)
        yt_c[:nr] = yt_cmp[lo:hi]
        yp_c = np.zeros((r_pad, c), dtype=np.float32)
        yp_c[:nr] = yp[lo:hi]
        ax_c = np.zeros((r_pad, 1), dtype=np.float32)
        ax_c[:nr, 0] = aux[lo:hi]
        in_maps.append({"yt": yt_c, "ypal": yp_c, "aux": ax_c})

    res = run_bass_kernel_spmd(nc, in_maps, list(range(N_CORES)))
    _last_results = res

    num = 0.0
    k = 0.0
    for i in range(N_CORES):
        p = res.results[i]["partials"]
        num += float(p[:, 0].sum(dtype=np.float64))
        k += float(p[:, 1].sum(dtype=np.float64))
    return np.float32(-num / k)
